# revision 5
# baseline (speedup 1.0000x reference)
import sys

sys.path.insert(0, "/opt/trn_rl_repo")

import ml_dtypes
import numpy as np

from concourse import bass, tile, bacc
from concourse.bass_utils import run_bass_kernel_spmd

WORLD, M, N, K_LOCAL = 8, 8192, 2048, 256
KT = WORLD * K_LOCAL  # 2048 — full contraction dim (K concatenated over ranks)
M_PER = M // WORLD  # 1024 output rows owned per core
KC = KT // 128  # 16 contraction chunks of 128
NI = N // 512  # 4 output column tiles of 512
MI = M_PER // 128  # 8 output row tiles of 128
BF16 = bass.mybir.dt.bfloat16
F32 = bass.mybir.dt.float32

LAST_RESULTS = None


def _build():
    # Each core computes its own [M_PER, N] output block over the full K:
    # out = At.T @ Wt with At [KT, M_PER], Wt [KT, N] — no collective needed.
    nc = bacc.Bacc(None, target_bir_lowering=False, num_devices=WORLD)
    At = nc.dram_tensor("At", [KT, M_PER], BF16, kind="ExternalInput")
    Wt = nc.dram_tensor("Wt", [KT, N], BF16, kind="ExternalInput")
    out = nc.dram_tensor("out", [M_PER, N], F32, kind="ExternalOutput")

    with tile.TileContext(nc) as tc:
        with (
            tc.tile_pool(name="resident", bufs=1) as res,
            tc.tile_pool(name="stage", bufs=8) as stage,
            tc.tile_pool(name="ps", bufs=8, space=bass.MemorySpace.PSUM) as ps,
        ):
            a_sb = res.tile([128, KC, M_PER], BF16)
            w_sb = res.tile([128, KC, N], BF16)
            # Wave ni=0 consumes A[kc] + W[kc, 0:512] per kc step, so issue
            # those first (interleaved per kc) and the remaining W slices after
            # — the PE starts after the first pair lands and never starves.
            for kc in range(KC):
                nc.sync.dma_start(a_sb[:, kc, :], At[kc * 128 : (kc + 1) * 128, :])
                nc.sync.dma_start(
                    w_sb[:, kc, 0:512], Wt[kc * 128 : (kc + 1) * 128, 0:512]
                )
            for ni in range(1, NI):
                for kc in range(KC):
                    nc.sync.dma_start(
                        w_sb[:, kc, ni * 512 : (ni + 1) * 512],
                        Wt[kc * 128 : (kc + 1) * 128, ni * 512 : (ni + 1) * 512],
                    )

            for ni in range(NI):
                # One wave = 8 PSUM banks, one per mi. k-outer matmul order
                # matches DMA arrival order so the PE tracks the loads.
                accs = [
                    ps.tile([128, 512], F32, name=f"acc_{ni}_{mi}", tag="acc")
                    for mi in range(MI)
                ]
                for kc in range(KC):
                    for mi in range(MI):
                        nc.tensor.matmul(
                            accs[mi][:],
                            a_sb[:, kc, mi * 128 : (mi + 1) * 128],
                            w_sb[:, kc, ni * 512 : (ni + 1) * 512],
                            start=(kc == 0),
                            stop=(kc == KC - 1),
                        )
                for mi in range(MI):
                    rowt = stage.tile([128, 512], F32)
                    # Alternate engines so wave-end PSUM drains don't serialize.
                    if mi % 2 == 0:
                        nc.vector.tensor_copy(rowt[:], accs[mi][:])
                    else:
                        nc.scalar.copy(rowt[:], accs[mi][:])
                    nc.sync.dma_start(
                        out[mi * 128 : (mi + 1) * 128, ni * 512 : (ni + 1) * 512],
                        rowt[:],
                    )
    nc.compile()
    return nc


def _in_maps(A, weight):
    A = np.asarray(A, dtype=np.float32)
    W = np.asarray(weight, dtype=np.float32)
    # [r, m, k] -> [r*K_LOCAL + k, m]: concatenate the per-rank K slices into
    # one contraction dim, pre-transposed so device DMAs are dense.
    At_full = A.transpose(0, 2, 1).reshape(KT, M).astype(ml_dtypes.bfloat16)
    Wt_full = np.ascontiguousarray(
        W.transpose(0, 2, 1).reshape(KT, N).astype(ml_dtypes.bfloat16)
    )
    return [
        {
            "At": np.ascontiguousarray(At_full[:, c * M_PER : (c + 1) * M_PER]),
            "Wt": Wt_full,
        }
        for c in range(WORLD)
    ]


def kernel(A, weight):
    nc = _build()
    in_maps = _in_maps(A, weight)
    res = run_bass_kernel_spmd(nc, in_maps, core_ids=list(range(WORLD)))
    global LAST_RESULTS
    LAST_RESULTS = res
    return np.stack(
        [np.asarray(res.results[c]["out"], dtype=np.float32) for c in range(WORLD)],
        axis=0,
    )


# revision 12
# speedup vs baseline: 9.5675x; 9.5675x over previous
import os
import sys

sys.path.insert(0, "/opt/trn_rl_repo")

import ml_dtypes
import numpy as np

try:  # pragma: no cover
    from antenv.axon_hooks import get_axon_ntff_profile_hook  # noqa: F401
except Exception:
    # Containers without the NTFF hook crash inside run_bass_kernel_spmd if
    # BASS_TRACE is set; disable tracing there rather than crash.
    os.environ["BASS_NEVER_TRACE"] = "1"

from concourse import bass, tile, bacc
from concourse.bass_utils import run_bass_kernel_spmd

WORLD, M, N, K_LOCAL = 8, 8192, 2048, 256
KT = WORLD * K_LOCAL  # 2048 — full contraction dim (K concatenated over ranks)
M_PER = M // WORLD  # 1024 output rows owned per core
KC = KT // 128  # 16 contraction chunks of 128
NI = N // 512  # 4 output column tiles of 512
MI = M_PER // 128  # 8 output row tiles of 128
BF16 = bass.mybir.dt.bfloat16
F32 = bass.mybir.dt.float32

LAST_RESULTS = None


def _build(repeats=1, loop_reps=0, wave="ni", dma_in_loop=True):
    # Each core computes its own [M_PER, N] output block over the full K:
    # out = At.T @ Wt with At [KT, M_PER], Wt [KT, N] — no collective needed.
    # repeats>1 / loop_reps>0 are timing-harness builds (serial body repeats,
    # the latter via a hardware For_i loop).
    # wave="ni":     4 waves over ni, 8 banks = 8 mi tiles, 1 LDW per MM.
    # wave="mi2ni4": 4 waves over mi-pairs, 8 banks = 2 mi x 4 ni, the 4 ni
    #                MMs share one stationary load (LDW amortization).
    nc = bacc.Bacc(None, target_bir_lowering=False, num_devices=WORLD)
    At = nc.dram_tensor("At", [KT, M_PER], BF16, kind="ExternalInput")
    Wt = nc.dram_tensor("Wt", [KT, N], BF16, kind="ExternalInput")
    out = nc.dram_tensor("out", [M_PER, N], F32, kind="ExternalOutput")

    with tile.TileContext(nc) as tc:
        with (
            tc.tile_pool(name="resident", bufs=1) as res,
            tc.tile_pool(name="stage", bufs=8) as stage,
            tc.tile_pool(name="ps", bufs=8, space=bass.MemorySpace.PSUM) as ps,
        ):

            def load_inputs(rep):
                a_sb = res.tile(
                    [128, KC, M_PER], BF16, name=f"a_sb_{rep}", tag="a_sb"
                )
                w_sb = res.tile([128, KC, N], BF16, name=f"w_sb_{rep}", tag="w_sb")
                if wave == "ni":
                    # ni=0 wave consumes A[kc] + W[kc, 0:512] per kc step:
                    # issue those first, remaining W slices after.
                    for kc in range(KC):
                        nc.sync.dma_start(
                            a_sb[:, kc, :], At[kc * 128 : (kc + 1) * 128, :]
                        )
                        nc.sync.dma_start(
                            w_sb[:, kc, 0:512], Wt[kc * 128 : (kc + 1) * 128, 0:512]
                        )
                    for ni in range(1, NI):
                        for kc in range(KC):
                            nc.sync.dma_start(
                                w_sb[:, kc, ni * 512 : (ni + 1) * 512],
                                Wt[kc * 128 : (kc + 1) * 128, ni * 512 : (ni + 1) * 512],
                            )
                elif wave == "kinner":
                    # First tile is (mi=0, ni=0) and consumes a[kc, mi=0] +
                    # w[kc, ni=0] for all kc — interleave those first so it
                    # can chase the DMAs, then the rest in consumption order.
                    for kc in range(KC):
                        nc.sync.dma_start(
                            a_sb[:, kc, 0:128], At[kc * 128 : (kc + 1) * 128, 0:128]
                        )
                        nc.sync.dma_start(
                            w_sb[:, kc, 0:512], Wt[kc * 128 : (kc + 1) * 128, 0:512]
                        )
                    for mi in range(1, MI):
                        for kc in range(KC):
                            nc.sync.dma_start(
                                a_sb[:, kc, mi * 128 : (mi + 1) * 128],
                                At[kc * 128 : (kc + 1) * 128, mi * 128 : (mi + 1) * 128],
                            )
                    for ni in range(1, NI):
                        for kc in range(KC):
                            nc.sync.dma_start(
                                w_sb[:, kc, ni * 512 : (ni + 1) * 512],
                                Wt[kc * 128 : (kc + 1) * 128, ni * 512 : (ni + 1) * 512],
                            )
                else:
                    # mi-pair waves need all of W[kc] per kc step.
                    for kc in range(KC):
                        nc.sync.dma_start(
                            a_sb[:, kc, :], At[kc * 128 : (kc + 1) * 128, :]
                        )
                        nc.sync.dma_start(
                            w_sb[:, kc, :], Wt[kc * 128 : (kc + 1) * 128, :]
                        )
                return a_sb, w_sb

            def compute(rep, a_sb, w_sb):
                if wave == "kinner":
                    # k-contiguous: 16 consecutive MMs into one PSUM bank per
                    # output tile (the production-kernel pattern — PE stays
                    # pipelined; bank cycling per-MM exposes ~170ns/MM).
                    t = 0
                    for ni in range(NI):
                        for mi in range(MI):
                            acc = ps.tile(
                                [128, 512], F32, name=f"acc_{rep}_{ni}_{mi}", tag="acc"
                            )
                            for kc in range(KC):
                                nc.tensor.matmul(
                                    acc[:],
                                    a_sb[:, kc, mi * 128 : (mi + 1) * 128],
                                    w_sb[:, kc, ni * 512 : (ni + 1) * 512],
                                    start=(kc == 0),
                                    stop=(kc == KC - 1),
                                )
                            rowt = stage.tile([128, 512], F32)
                            if t % 2 == 0:
                                nc.vector.tensor_copy(rowt[:], acc[:])
                            else:
                                nc.scalar.copy(rowt[:], acc[:])
                            nc.sync.dma_start(
                                out[
                                    mi * 128 : (mi + 1) * 128,
                                    ni * 512 : (ni + 1) * 512,
                                ],
                                rowt[:],
                            )
                            t += 1
                    return
                if wave == "ni":
                    for ni in range(NI):
                        accs = [
                            ps.tile(
                                [128, 512], F32, name=f"acc_{rep}_{ni}_{mi}", tag="acc"
                            )
                            for mi in range(MI)
                        ]
                        for kc in range(KC):
                            for mi in range(MI):
                                nc.tensor.matmul(
                                    accs[mi][:],
                                    a_sb[:, kc, mi * 128 : (mi + 1) * 128],
                                    w_sb[:, kc, ni * 512 : (ni + 1) * 512],
                                    start=(kc == 0),
                                    stop=(kc == KC - 1),
                                )
                        for mi in range(MI):
                            rowt = stage.tile([128, 512], F32)
                            if mi % 2 == 0:
                                nc.vector.tensor_copy(rowt[:], accs[mi][:])
                            else:
                                nc.scalar.copy(rowt[:], accs[mi][:])
                            nc.sync.dma_start(
                                out[
                                    mi * 128 : (mi + 1) * 128,
                                    ni * 512 : (ni + 1) * 512,
                                ],
                                rowt[:],
                            )
                else:
                    for wv in range(MI // 2):
                        accs = [
                            ps.tile(
                                [128, 512],
                                F32,
                                name=f"acc_{rep}_{wv}_{t}",
                                tag="acc",
                            )
                            for t in range(8)
                        ]
                        for kc in range(KC):
                            for m2 in range(2):
                                mi = wv * 2 + m2
                                for ni in range(NI):
                                    nc.tensor.matmul(
                                        accs[m2 * NI + ni][:],
                                        a_sb[:, kc, mi * 128 : (mi + 1) * 128],
                                        w_sb[:, kc, ni * 512 : (ni + 1) * 512],
                                        start=(kc == 0),
                                        stop=(kc == KC - 1),
                                    )
                        for m2 in range(2):
                            mi = wv * 2 + m2
                            for ni in range(NI):
                                rowt = stage.tile([128, 512], F32)
                                if ni % 2 == 0:
                                    nc.vector.tensor_copy(
                                        rowt[:], accs[m2 * NI + ni][:]
                                    )
                                else:
                                    nc.scalar.copy(rowt[:], accs[m2 * NI + ni][:])
                                nc.sync.dma_start(
                                    out[
                                        mi * 128 : (mi + 1) * 128,
                                        ni * 512 : (ni + 1) * 512,
                                    ],
                                    rowt[:],
                                )

            hoisted = None
            if not dma_in_loop:
                hoisted = load_inputs(0)
            loop_ctx = tc.For_i(0, loop_reps, 1) if loop_reps else None
            if loop_ctx is not None:
                loop_ctx.__enter__()
            for rep in range(repeats):
                if hoisted is None:
                    a_sb, w_sb = load_inputs(rep)
                else:
                    a_sb, w_sb = hoisted
                compute(rep, a_sb, w_sb)
            if loop_ctx is not None:
                loop_ctx.__exit__(None, None, None)
    nc.compile()
    return nc


def _in_maps(A, weight):
    A = np.asarray(A, dtype=np.float32)
    W = np.asarray(weight, dtype=np.float32)
    # [r, m, k] -> [r*K_LOCAL + k, m]: concatenate the per-rank K slices into
    # one contraction dim, pre-transposed so device DMAs are dense.
    At_full = A.transpose(0, 2, 1).reshape(KT, M).astype(ml_dtypes.bfloat16)
    Wt_full = np.ascontiguousarray(
        W.transpose(0, 2, 1).reshape(KT, N).astype(ml_dtypes.bfloat16)
    )
    return [
        {
            "At": np.ascontiguousarray(At_full[:, c * M_PER : (c + 1) * M_PER]),
            "Wt": Wt_full,
        }
        for c in range(WORLD)
    ]


def kernel(A, weight):
    nc = _build()
    in_maps = _in_maps(A, weight)
    res = run_bass_kernel_spmd(nc, in_maps, core_ids=list(range(WORLD)))
    global LAST_RESULTS
    LAST_RESULTS = res
    return np.stack(
        [np.asarray(res.results[c]["out"], dtype=np.float32) for c in range(WORLD)],
        axis=0,
    )


# revision 14
# speedup vs baseline: 9.6004x; 1.0034x over previous
import os
import sys

sys.path.insert(0, "/opt/trn_rl_repo")

import ml_dtypes
import numpy as np

try:  # pragma: no cover
    from antenv.axon_hooks import get_axon_ntff_profile_hook  # noqa: F401
except Exception:
    # Containers without the NTFF hook crash inside run_bass_kernel_spmd if
    # BASS_TRACE is set; disable tracing there rather than crash.
    os.environ["BASS_NEVER_TRACE"] = "1"

from concourse import bass, tile, bacc
from concourse.bass_utils import run_bass_kernel_spmd

WORLD, M, N, K_LOCAL = 8, 8192, 2048, 256
KT = WORLD * K_LOCAL  # 2048 — full contraction dim (K concatenated over ranks)
M_PER = M // WORLD  # 1024 output rows owned per core
KC = KT // 128  # 16 contraction chunks of 128
NI = N // 512  # 4 output column tiles of 512
MI = M_PER // 128  # 8 output row tiles of 128
BF16 = bass.mybir.dt.bfloat16
F32 = bass.mybir.dt.float32

LAST_RESULTS = None


def _build(repeats=1, loop_reps=0, wave="ni", dma_in_loop=True):
    # Each core computes its own [M_PER, N] output block over the full K:
    # out = At.T @ Wt with At [KT, M_PER], Wt [KT, N] — no collective needed.
    # repeats>1 / loop_reps>0 are timing-harness builds (serial body repeats,
    # the latter via a hardware For_i loop).
    # wave="ni":     4 waves over ni, 8 banks = 8 mi tiles, 1 LDW per MM.
    # wave="mi2ni4": 4 waves over mi-pairs, 8 banks = 2 mi x 4 ni, the 4 ni
    #                MMs share one stationary load (LDW amortization).
    nc = bacc.Bacc(None, target_bir_lowering=False, num_devices=WORLD)
    At = nc.dram_tensor("At", [KT, M_PER], BF16, kind="ExternalInput")
    Wt = nc.dram_tensor("Wt", [KT, N], BF16, kind="ExternalInput")
    out = nc.dram_tensor("out", [M_PER, N], F32, kind="ExternalOutput")

    with tile.TileContext(nc) as tc:
        with (
            tc.tile_pool(name="resident", bufs=1) as res,
            tc.tile_pool(name="stage", bufs=8) as stage,
            tc.tile_pool(name="ps", bufs=8, space=bass.MemorySpace.PSUM) as ps,
        ):

            def load_inputs(rep):
                a_sb = res.tile(
                    [128, KC, M_PER], BF16, name=f"a_sb_{rep}", tag="a_sb"
                )
                w_sb = res.tile([128, KC, N], BF16, name=f"w_sb_{rep}", tag="w_sb")
                if wave == "ni":
                    # ni=0 wave consumes A[kc] + W[kc, 0:512] per kc step:
                    # issue those first, remaining W slices after.
                    for kc in range(KC):
                        nc.sync.dma_start(
                            a_sb[:, kc, :], At[kc * 128 : (kc + 1) * 128, :]
                        )
                        nc.sync.dma_start(
                            w_sb[:, kc, 0:512], Wt[kc * 128 : (kc + 1) * 128, 0:512]
                        )
                    for ni in range(1, NI):
                        for kc in range(KC):
                            nc.sync.dma_start(
                                w_sb[:, kc, ni * 512 : (ni + 1) * 512],
                                Wt[kc * 128 : (kc + 1) * 128, ni * 512 : (ni + 1) * 512],
                            )
                elif wave == "kinner":
                    # First tile is (mi=0, ni=0) and consumes a[kc, mi=0] +
                    # w[kc, ni=0] for all kc — interleave those first so it
                    # can chase the DMAs, then the rest in consumption order.
                    for kc in range(KC):
                        nc.sync.dma_start(
                            a_sb[:, kc, 0:128], At[kc * 128 : (kc + 1) * 128, 0:128]
                        )
                        nc.sync.dma_start(
                            w_sb[:, kc, 0:512], Wt[kc * 128 : (kc + 1) * 128, 0:512]
                        )
                    for mi in range(1, MI):
                        for kc in range(KC):
                            nc.sync.dma_start(
                                a_sb[:, kc, mi * 128 : (mi + 1) * 128],
                                At[kc * 128 : (kc + 1) * 128, mi * 128 : (mi + 1) * 128],
                            )
                    for ni in range(1, NI):
                        for kc in range(KC):
                            nc.sync.dma_start(
                                w_sb[:, kc, ni * 512 : (ni + 1) * 512],
                                Wt[kc * 128 : (kc + 1) * 128, ni * 512 : (ni + 1) * 512],
                            )
                else:
                    # mi-pair waves need all of W[kc] per kc step.
                    for kc in range(KC):
                        nc.sync.dma_start(
                            a_sb[:, kc, :], At[kc * 128 : (kc + 1) * 128, :]
                        )
                        nc.sync.dma_start(
                            w_sb[:, kc, :], Wt[kc * 128 : (kc + 1) * 128, :]
                        )
                return a_sb, w_sb

            def compute(rep, a_sb, w_sb):
                if wave == "kinner":
                    # k-contiguous: 16 consecutive MMs into one PSUM bank per
                    # output tile (the production-kernel pattern — PE stays
                    # pipelined; bank cycling per-MM exposes ~170ns/MM).
                    t = 0
                    for ni in range(NI):
                        for mi in range(MI):
                            acc = ps.tile(
                                [128, 512], F32, name=f"acc_{rep}_{ni}_{mi}", tag="acc"
                            )
                            for kc in range(KC):
                                nc.tensor.matmul(
                                    acc[:],
                                    a_sb[:, kc, mi * 128 : (mi + 1) * 128],
                                    w_sb[:, kc, ni * 512 : (ni + 1) * 512],
                                    start=(kc == 0),
                                    stop=(kc == KC - 1),
                                )
                            rowt = stage.tile([128, 512], F32)
                            if t % 2 == 0:
                                nc.vector.tensor_copy(rowt[:], acc[:])
                            else:
                                nc.scalar.copy(rowt[:], acc[:])
                            nc.sync.dma_start(
                                out[
                                    mi * 128 : (mi + 1) * 128,
                                    ni * 512 : (ni + 1) * 512,
                                ],
                                rowt[:],
                            )
                            t += 1
                    return
                if wave in ("ni", "col2", "col4"):
                    # col2/col4: split each MM into 2/4 column-group MMs
                    # (M=64/32 slices). Output slices at base partitions
                    # 0/32/64/96 auto-derive tile_position col groups; the
                    # smaller LDWEIGHTS (P=64/32 cols) can pull ahead while
                    # sibling col-groups stream, and the sibling MMs run
                    # concurrently in disjoint PE column strips.
                    nsplit = {"ni": 1, "col2": 2, "col4": 4}[wave]
                    mstep = 128 // nsplit
                    for ni in range(NI):
                        accs = [
                            ps.tile(
                                [128, 512], F32, name=f"acc_{rep}_{ni}_{mi}", tag="acc"
                            )
                            for mi in range(MI)
                        ]
                        for kc in range(KC):
                            for mi in range(MI):
                                for s in range(nsplit):
                                    nc.tensor.matmul(
                                        accs[mi][s * mstep : (s + 1) * mstep, :],
                                        a_sb[
                                            :,
                                            kc,
                                            mi * 128 + s * mstep : mi * 128
                                            + (s + 1) * mstep,
                                        ],
                                        w_sb[:, kc, ni * 512 : (ni + 1) * 512],
                                        start=(kc == 0),
                                        stop=(kc == KC - 1),
                                        tile_position=(0, s * mstep)
                                        if nsplit > 1
                                        else None,
                                    )
                        for mi in range(MI):
                            rowt = stage.tile([128, 512], F32)
                            if mi % 2 == 0:
                                nc.vector.tensor_copy(rowt[:], accs[mi][:])
                            else:
                                nc.scalar.copy(rowt[:], accs[mi][:])
                            nc.sync.dma_start(
                                out[
                                    mi * 128 : (mi + 1) * 128,
                                    ni * 512 : (ni + 1) * 512,
                                ],
                                rowt[:],
                            )
                else:
                    for wv in range(MI // 2):
                        accs = [
                            ps.tile(
                                [128, 512],
                                F32,
                                name=f"acc_{rep}_{wv}_{t}",
                                tag="acc",
                            )
                            for t in range(8)
                        ]
                        for kc in range(KC):
                            for m2 in range(2):
                                mi = wv * 2 + m2
                                for ni in range(NI):
                                    nc.tensor.matmul(
                                        accs[m2 * NI + ni][:],
                                        a_sb[:, kc, mi * 128 : (mi + 1) * 128],
                                        w_sb[:, kc, ni * 512 : (ni + 1) * 512],
                                        start=(kc == 0),
                                        stop=(kc == KC - 1),
                                    )
                        for m2 in range(2):
                            mi = wv * 2 + m2
                            for ni in range(NI):
                                rowt = stage.tile([128, 512], F32)
                                if ni % 2 == 0:
                                    nc.vector.tensor_copy(
                                        rowt[:], accs[m2 * NI + ni][:]
                                    )
                                else:
                                    nc.scalar.copy(rowt[:], accs[m2 * NI + ni][:])
                                nc.sync.dma_start(
                                    out[
                                        mi * 128 : (mi + 1) * 128,
                                        ni * 512 : (ni + 1) * 512,
                                    ],
                                    rowt[:],
                                )

            hoisted = None
            if not dma_in_loop:
                hoisted = load_inputs(0)
            loop_ctx = tc.For_i(0, loop_reps, 1) if loop_reps else None
            if loop_ctx is not None:
                loop_ctx.__enter__()
            for rep in range(repeats):
                if hoisted is None:
                    a_sb, w_sb = load_inputs(rep)
                else:
                    a_sb, w_sb = hoisted
                compute(rep, a_sb, w_sb)
            if loop_ctx is not None:
                loop_ctx.__exit__(None, None, None)
    nc.compile()
    return nc


def _in_maps(A, weight):
    A = np.asarray(A, dtype=np.float32)
    W = np.asarray(weight, dtype=np.float32)
    # [r, m, k] -> [r*K_LOCAL + k, m]: concatenate the per-rank K slices into
    # one contraction dim, pre-transposed so device DMAs are dense.
    At_full = A.transpose(0, 2, 1).reshape(KT, M).astype(ml_dtypes.bfloat16)
    Wt_full = np.ascontiguousarray(
        W.transpose(0, 2, 1).reshape(KT, N).astype(ml_dtypes.bfloat16)
    )
    return [
        {
            "At": np.ascontiguousarray(At_full[:, c * M_PER : (c + 1) * M_PER]),
            "Wt": Wt_full,
        }
        for c in range(WORLD)
    ]


def kernel(A, weight):
    nc = _build()
    in_maps = _in_maps(A, weight)
    res = run_bass_kernel_spmd(nc, in_maps, core_ids=list(range(WORLD)))
    global LAST_RESULTS
    LAST_RESULTS = res
    return np.stack(
        [np.asarray(res.results[c]["out"], dtype=np.float32) for c in range(WORLD)],
        axis=0,
    )


# revision 17
# speedup vs baseline: 9.6839x; 1.0087x over previous
import os
import sys

sys.path.insert(0, "/opt/trn_rl_repo")

import ml_dtypes
import numpy as np

try:  # pragma: no cover
    from antenv.axon_hooks import get_axon_ntff_profile_hook  # noqa: F401
except Exception:
    # Containers without the NTFF hook crash inside run_bass_kernel_spmd if
    # BASS_TRACE is set; disable tracing there rather than crash.
    os.environ["BASS_NEVER_TRACE"] = "1"

from concourse import bass, tile, bacc
from concourse.bass_utils import run_bass_kernel_spmd

WORLD, M, N, K_LOCAL = 8, 8192, 2048, 256
KT = WORLD * K_LOCAL  # 2048 — full contraction dim (K concatenated over ranks)
M_PER = M // WORLD  # 1024 output rows owned per core
KC = KT // 128  # 16 contraction chunks of 128
NI = N // 512  # 4 output column tiles of 512
MI = M_PER // 128  # 8 output row tiles of 128
BF16 = bass.mybir.dt.bfloat16
F32 = bass.mybir.dt.float32

LAST_RESULTS = None


def _build(repeats=1, loop_reps=0, wave="ni", dma_in_loop=True):
    # Each core computes its own [M_PER, N] output block over the full K:
    # out = At.T @ Wt with At [KT, M_PER], Wt [KT, N] — no collective needed.
    # repeats>1 / loop_reps>0 are timing-harness builds (serial body repeats,
    # the latter via a hardware For_i loop).
    # wave="ni":     4 waves over ni, 8 banks = 8 mi tiles, 1 LDW per MM.
    # wave="mi2ni4": 4 waves over mi-pairs, 8 banks = 2 mi x 4 ni, the 4 ni
    #                MMs share one stationary load (LDW amortization).
    nc = bacc.Bacc(None, target_bir_lowering=False, num_devices=WORLD)
    At = nc.dram_tensor("At", [KT, M_PER], BF16, kind="ExternalInput")
    Wt = nc.dram_tensor("Wt", [KT, N], BF16, kind="ExternalInput")
    out = nc.dram_tensor("out", [M_PER, N], F32, kind="ExternalOutput")

    with tile.TileContext(nc) as tc:
        with (
            tc.tile_pool(name="resident", bufs=1) as res,
            tc.tile_pool(name="stage", bufs=8) as stage,
            tc.tile_pool(name="ps", bufs=8, space=bass.MemorySpace.PSUM) as ps,
        ):

            def load_inputs(rep):
                a_sb = res.tile(
                    [128, KC, M_PER], BF16, name=f"a_sb_{rep}", tag="a_sb"
                )
                w_sb = res.tile([128, KC, N], BF16, name=f"w_sb_{rep}", tag="w_sb")
                if wave == "ni":
                    # ni=0 wave consumes A[kc] + W[kc, 0:512] per kc step:
                    # issue those first, remaining W slices after.
                    for kc in range(KC):
                        nc.sync.dma_start(
                            a_sb[:, kc, :], At[kc * 128 : (kc + 1) * 128, :]
                        )
                        nc.sync.dma_start(
                            w_sb[:, kc, 0:512], Wt[kc * 128 : (kc + 1) * 128, 0:512]
                        )
                    for ni in range(1, NI):
                        for kc in range(KC):
                            nc.sync.dma_start(
                                w_sb[:, kc, ni * 512 : (ni + 1) * 512],
                                Wt[kc * 128 : (kc + 1) * 128, ni * 512 : (ni + 1) * 512],
                            )
                elif wave == "kinner":
                    # First tile is (mi=0, ni=0) and consumes a[kc, mi=0] +
                    # w[kc, ni=0] for all kc — interleave those first so it
                    # can chase the DMAs, then the rest in consumption order.
                    for kc in range(KC):
                        nc.sync.dma_start(
                            a_sb[:, kc, 0:128], At[kc * 128 : (kc + 1) * 128, 0:128]
                        )
                        nc.sync.dma_start(
                            w_sb[:, kc, 0:512], Wt[kc * 128 : (kc + 1) * 128, 0:512]
                        )
                    for mi in range(1, MI):
                        for kc in range(KC):
                            nc.sync.dma_start(
                                a_sb[:, kc, mi * 128 : (mi + 1) * 128],
                                At[kc * 128 : (kc + 1) * 128, mi * 128 : (mi + 1) * 128],
                            )
                    for ni in range(1, NI):
                        for kc in range(KC):
                            nc.sync.dma_start(
                                w_sb[:, kc, ni * 512 : (ni + 1) * 512],
                                Wt[kc * 128 : (kc + 1) * 128, ni * 512 : (ni + 1) * 512],
                            )
                else:
                    # mi-pair waves need all of W[kc] per kc step.
                    for kc in range(KC):
                        nc.sync.dma_start(
                            a_sb[:, kc, :], At[kc * 128 : (kc + 1) * 128, :]
                        )
                        nc.sync.dma_start(
                            w_sb[:, kc, :], Wt[kc * 128 : (kc + 1) * 128, :]
                        )
                return a_sb, w_sb

            def compute(rep, a_sb, w_sb):
                if wave == "kinner":
                    # k-contiguous: 16 consecutive MMs into one PSUM bank per
                    # output tile (the production-kernel pattern — PE stays
                    # pipelined; bank cycling per-MM exposes ~170ns/MM).
                    t = 0
                    for ni in range(NI):
                        for mi in range(MI):
                            acc = ps.tile(
                                [128, 512], F32, name=f"acc_{rep}_{ni}_{mi}", tag="acc"
                            )
                            for kc in range(KC):
                                nc.tensor.matmul(
                                    acc[:],
                                    a_sb[:, kc, mi * 128 : (mi + 1) * 128],
                                    w_sb[:, kc, ni * 512 : (ni + 1) * 512],
                                    start=(kc == 0),
                                    stop=(kc == KC - 1),
                                )
                            rowt = stage.tile([128, 512], F32)
                            if t % 2 == 0:
                                nc.vector.tensor_copy(rowt[:], acc[:])
                            else:
                                nc.scalar.copy(rowt[:], acc[:])
                            nc.sync.dma_start(
                                out[
                                    mi * 128 : (mi + 1) * 128,
                                    ni * 512 : (ni + 1) * 512,
                                ],
                                rowt[:],
                            )
                            t += 1
                    return
                if wave in ("ni", "col2", "col4"):
                    # col2/col4: split each MM into 2/4 column-group MMs
                    # (M=64/32 slices). Output slices at base partitions
                    # 0/32/64/96 auto-derive tile_position col groups; the
                    # smaller LDWEIGHTS (P=64/32 cols) can pull ahead while
                    # sibling col-groups stream, and the sibling MMs run
                    # concurrently in disjoint PE column strips.
                    nsplit = {"ni": 1, "col2": 2, "col4": 4}[wave]
                    mstep = 128 // nsplit
                    for ni in range(NI):
                        accs = [
                            ps.tile(
                                [128, 512], F32, name=f"acc_{rep}_{ni}_{mi}", tag="acc"
                            )
                            for mi in range(MI)
                        ]
                        for kc in range(KC):
                            for mi in range(MI):
                                for s in range(nsplit):
                                    nc.tensor.matmul(
                                        accs[mi][s * mstep : (s + 1) * mstep, :],
                                        a_sb[
                                            :,
                                            kc,
                                            mi * 128 + s * mstep : mi * 128
                                            + (s + 1) * mstep,
                                        ],
                                        w_sb[:, kc, ni * 512 : (ni + 1) * 512],
                                        start=(kc == 0),
                                        stop=(kc == KC - 1),
                                        tile_position=(0, s * mstep)
                                        if nsplit > 1
                                        else None,
                                    )
                        for mi in range(MI):
                            rowt = stage.tile([128, 512], F32)
                            if mi % 2 == 0:
                                nc.vector.tensor_copy(rowt[:], accs[mi][:])
                            else:
                                nc.scalar.copy(rowt[:], accs[mi][:])
                            nc.sync.dma_start(
                                out[
                                    mi * 128 : (mi + 1) * 128,
                                    ni * 512 : (ni + 1) * 512,
                                ],
                                rowt[:],
                            )
                else:
                    for wv in range(MI // 2):
                        accs = [
                            ps.tile(
                                [128, 512],
                                F32,
                                name=f"acc_{rep}_{wv}_{t}",
                                tag="acc",
                            )
                            for t in range(8)
                        ]
                        for kc in range(KC):
                            for m2 in range(2):
                                mi = wv * 2 + m2
                                for ni in range(NI):
                                    nc.tensor.matmul(
                                        accs[m2 * NI + ni][:],
                                        a_sb[:, kc, mi * 128 : (mi + 1) * 128],
                                        w_sb[:, kc, ni * 512 : (ni + 1) * 512],
                                        start=(kc == 0),
                                        stop=(kc == KC - 1),
                                    )
                        for m2 in range(2):
                            mi = wv * 2 + m2
                            for ni in range(NI):
                                rowt = stage.tile([128, 512], F32)
                                if ni % 2 == 0:
                                    nc.vector.tensor_copy(
                                        rowt[:], accs[m2 * NI + ni][:]
                                    )
                                else:
                                    nc.scalar.copy(rowt[:], accs[m2 * NI + ni][:])
                                nc.sync.dma_start(
                                    out[
                                        mi * 128 : (mi + 1) * 128,
                                        ni * 512 : (ni + 1) * 512,
                                    ],
                                    rowt[:],
                                )

            hoisted = None
            if not dma_in_loop:
                hoisted = load_inputs(0)
            loop_ctx = tc.For_i(0, loop_reps, 1) if loop_reps else None
            if loop_ctx is not None:
                loop_ctx.__enter__()
            for rep in range(repeats):
                if hoisted is None:
                    a_sb, w_sb = load_inputs(rep)
                else:
                    a_sb, w_sb = hoisted
                compute(rep, a_sb, w_sb)
            if loop_ctx is not None:
                loop_ctx.__exit__(None, None, None)
    nc.compile()
    return nc


def _in_maps(A, weight):
    A = np.asarray(A, dtype=np.float32)
    W = np.asarray(weight, dtype=np.float32)
    # [r, m, k] -> [r*K_LOCAL + k, m]: concatenate the per-rank K slices into
    # one contraction dim, pre-transposed so device DMAs are dense.
    At_full = A.transpose(0, 2, 1).reshape(KT, M).astype(ml_dtypes.bfloat16)
    Wt_full = np.ascontiguousarray(
        W.transpose(0, 2, 1).reshape(KT, N).astype(ml_dtypes.bfloat16)
    )
    return [
        {
            "At": np.ascontiguousarray(At_full[:, c * M_PER : (c + 1) * M_PER]),
            "Wt": Wt_full,
        }
        for c in range(WORLD)
    ]


def kernel(A, weight):
    nc = _build()
    in_maps = _in_maps(A, weight)
    res = run_bass_kernel_spmd(nc, in_maps, core_ids=list(range(WORLD)))
    global LAST_RESULTS
    LAST_RESULTS = res
    return np.stack(
        [np.asarray(res.results[c]["out"], dtype=np.float32) for c in range(WORLD)],
        axis=0,
    )
